# revision 17
# baseline (speedup 1.0000x reference)
"""2D DCT-II (ortho) on (32, 3, 512, 512) fp32, data-parallel across 8 TRN2 NeuronCores.

out = D @ X @ D.T per image, with the W axis folded by the DCT symmetry
D[k, 511-w] = (-1)^k D[k, w]:
  EW[h, w'] = X[h, w'] + X[h, 511-w'],  OW[h, w'] = X[h, w'] - X[h, 511-w']
  pass A (H-DCT): P1e[w', k] = sum_h EW[h, w'] D.T[h, k]   (lhsT=EW, rhs=D.T)
                  16 MMs x 512 free, contraction 4x128
  pass B (W-DCT): OUT[k, 2j+b] via P1{e,o} against De/Do^T
                  16 MMs x 256 free, contraction 2x128
This MM shape measured 2.19 rows/ns on HW (512-free MMs hide the LDWEIGHTS
overhead; an H-folded variant with all-256-free MMs only reaches 1.46).
PE is the bottleneck stage at ~5.6us/image vs DMA 5.2, so the schedule is
software-pipelined: image i's loads+folds are queued BEFORE image i-1's
passes, giving the fold chain a full image-period of slack; 14 warmup
matmuls hold the HAM clock gate at 8/8 through the DMA ramp; drains and
stores are split across DVE/ACT/GPSIMD/SP so no helper engine exceeds
~4.5us/image.
"""
import os
import sys

for _p in ("/opt/trn_rl_repo", os.path.expanduser("~/.axon_site/_ro/trn_rl_repo")):
    if os.path.isdir(_p) and _p not in sys.path:
        sys.path.insert(0, _p)

import numpy as np
import concourse.bass as bass
import concourse.bacc as bacc
import concourse.mybir as mybir
import concourse.tile as tile
from concourse.bass_utils import run_bass_kernel_spmd

dt = mybir.dt

N = 512            # image height/width
H = N // 2         # 256, folded width
P = 128            # SBUF partitions
N_CORES = 8
B, CH = 32, 3      # full input batch/channels
IMGS = (B * CH) // N_CORES  # 12 images per core


def _dct_matrix() -> np.ndarray:
    n = np.arange(N, dtype=np.float64)
    k = n[:, None]
    D = np.cos(np.pi * (2.0 * n[None, :] + 1.0) * k / (2.0 * N))
    D[0] *= np.sqrt(1.0 / N)
    D[1:] *= np.sqrt(2.0 / N)
    return D


def _consts() -> tuple[np.ndarray, np.ndarray]:
    from ml_dtypes import bfloat16

    D = _dct_matrix()
    dct_t = np.ascontiguousarray(D.T.astype(bfloat16))              # [h, k]
    de_t = D[0::2, :H].T                                            # [w', j]
    do_t = D[1::2, :H].T
    deo = np.concatenate([de_t, do_t], axis=0).astype(bfloat16)     # [512, 256]
    return dct_t, np.ascontiguousarray(deo)


def _build_nc() -> bacc.Bacc:
    nc = bacc.Bacc("TRN2", target_bir_lowering=False, debug=False, num_devices=N_CORES)
    inp = nc.dram_tensor("inp", [IMGS, N, N], dt.float32, kind="ExternalInput")
    out = nc.dram_tensor("out", [IMGS, N, N], dt.float32, kind="ExternalOutput")
    dct_t = nc.dram_tensor("dct_t", [N, N], dt.bfloat16, kind="ExternalInput")
    deo_t = nc.dram_tensor("deo_t", [N, H], dt.bfloat16, kind="ExternalInput")

    bf16 = dt.bfloat16
    f32 = dt.float32
    ia = inp.ap()
    oa = out.ap()

    with tile.TileContext(nc) as tc:
        with (
            tc.tile_pool(name="const", bufs=1) as const_pool,
            tc.tile_pool(name="tin", bufs=5) as tin_pool,
            tc.tile_pool(name="quad", bufs=3) as quad_pool,
            tc.tile_pool(name="mid", bufs=3) as mid_pool,
            tc.tile_pool(name="res", bufs=2) as res_pool,
            tc.tile_pool(name="psA", bufs=2, space="PSUM") as psa_pool,
            tc.tile_pool(name="psB", bufs=4, space="PSUM") as psb_pool,
        ):
            # D.T resident in SBUF: dt_sb[p, c*512 + k] = D.T[128c+p, k]
            dt_c0 = const_pool.tile([P, N], bf16)
            nc.scalar.dma_start(dt_c0[:], dct_t.ap()[0:P, :])
            dt_r = const_pool.tile([P, 3 * N], bf16)
            nc.scalar.dma_start(
                dt_r[:].rearrange("p (c f) -> p c f", c=3),
                dct_t.ap()[P:, :].rearrange("(c p) f -> p c f", p=P),
            )

            def dt_slice(c):
                return dt_c0[:] if c == 0 else dt_r[:, N * (c - 1) : N * c]

            # deo_sb[p, 256*q + j] = deo[128q + p, j]; q=0,1 even k_w, 2,3 odd
            deo_sb = const_pool.tile([P, 2 * N], bf16)
            nc.scalar.dma_start(
                deo_sb[:].rearrange("p (q j) -> p q j", q=4),
                deo_t.ap().rearrange("(q p) j -> p q j", p=P),
            )

            # PE warmup across the DMA ramp + image-0 fold latency: dummy
            # matmuls flip the HAM clock gate to 8/8 before real work lands.
            scr_f = const_pool.tile([P, N + P], f32)
            nc.gpsimd.memset(scr_f[:], 0.0)
            scr = const_pool.tile([P, N + P], bf16)
            nc.vector.tensor_copy(scr[:], scr_f[:])
            ps_w = psb_pool.tile([P, N], f32, tag="psB")
            for _ in range(14):
                nc.tensor.matmul(
                    ps_w[:], scr[:, N : N + P], scr[:, :N], start=True, stop=True
                )

            state: dict[int, dict] = {}

            def load(i):
                # t_sb[p, c*512+w] = X[128c+p, w]; bn_sb: rows 256..511
                t_sb = tin_pool.tile([P, 2 * N], f32, tag="t", name=f"t_{i}")
                bn_sb = tin_pool.tile([P, 2 * N], f32, tag="bn", name=f"bn_{i}")
                nc.sync.dma_start(
                    t_sb[:].rearrange("p (c f) -> p c f", c=2),
                    ia[i][0 : 2 * P, :].rearrange("(c p) f -> p c f", p=P),
                )
                nc.sync.dma_start(
                    bn_sb[:].rearrange("p (c f) -> p c f", c=2),
                    ia[i][2 * P : 4 * P, :].rearrange("(c p) f -> p c f", p=P),
                )
                state[i] = {"t": t_sb, "bn": bn_sb}

            def folds(i):
                # EW/OW[p, c*256+w'] = X[.., w'] +/- X[.., 511-w'], split into
                # top/bottom-half tiles so pass A's accumulation can start
                # after the first fold op.  DVE folds the top half, GPSIMD
                # the bottom half (DVE both for the ramp-critical first two
                # images - GPSIMD ops have high dispatch latency).
                st = state[i]
                # emission order = pass A consumption order: ew_t, ew_b
                # (parity e, chunks 0-3), then ow_t, ow_b (parity o)
                for half, par in (("t", "e"), ("b", "e"), ("t", "o"), ("b", "o")):
                    src = st["t"] if half == "t" else st["bn"]
                    sa = src[:]
                    lo = bass.AP(
                        sa.tensor, sa.offset, [[sa.ap[0][0], P], [N, 2], [1, H]]
                    )
                    hi_rev = bass.AP(
                        sa.tensor, sa.offset + N - 1,
                        [[sa.ap[0][0], P], [N, 2], [-1, H]],
                    )
                    if True:
                        eng = (
                            nc.vector
                            if i < 2 or (half, par) == ("t", "e")
                            else nc.gpsimd
                        )
                        vop = eng.tensor_add if par == "e" else eng.tensor_sub
                        q = quad_pool.tile(
                            [P, 2 * H], bf16, tag=f"{par}w{half}", name=f"{par}w{half}_{i}"
                        )
                        vop(q[:].rearrange("p (c j) -> p c j", c=2), lo, hi_rev)
                        st[par + half] = q

            def passA(i):
                st = state[i]

                def fold_slice(par, c, col):
                    q = st[par + ("t" if c < 2 else "b")]
                    return q[:, (c % 2) * H + col : (c % 2) * H + col + P]

                # pass A (H-DCT): per parity, psum [128, 2 win x 512]
                p1 = {}
                for p_i, par in enumerate("eo"):
                    ps = psa_pool.tile([P, 2 * N], f32, tag="psA", name=f"psA{par}_{i}")
                    for m in range(2):      # w' window
                        for c in range(4):  # h chunk (accumulate)
                            nc.tensor.matmul(
                                ps[:, N * m : N * (m + 1)],
                                fold_slice(par, c, m * P),
                                dt_slice(c),
                                start=(c == 0), stop=(c == 3),
                            )
                    mt = mid_pool.tile([P, 2 * N], bf16, tag=f"p1{par}", name=f"p1{par}_{i}")
                    nc.scalar.copy(mt[:], ps[:])
                    p1[par] = mt
                st["p1"] = p1
                for k in ("t", "bn", "et", "ot", "eb", "ob"):
                    st.pop(k, None)

            def passB(i):
                st = state[i]
                p1 = st["p1"]
                # pass B (W-DCT): k_h windows m4; psum [128, 256b + j]
                o_half = [
                    res_pool.tile([P, 2 * N], f32, tag="o0", name=f"oh0_{i}"),
                    res_pool.tile([P, 2 * N], f32, tag="o1", name=f"oh1_{i}"),
                ]
                for m4 in range(4):
                    ps = psb_pool.tile([P, N], f32, tag="psB", name=f"psB{m4}_{i}")
                    for b_i in range(2):
                        for cw in range(2):  # w' chunk (accumulate)
                            nc.tensor.matmul(
                                ps[:, H * b_i : H * (b_i + 1)],
                                p1["eo"[b_i]][:, cw * N + m4 * P : cw * N + m4 * P + P],
                                deo_sb[:, H * (2 * b_i + cw) : H * (2 * b_i + cw + 1)],
                                start=(cw == 0), stop=(cw == 1),
                            )
                    # interleave drain (DVE): o[p, 512*(m4%2)+2j+b] = ps[p, 256b+j]
                    src = ps[:].rearrange("p (h j) -> p h j", h=2)
                    ob = o_half[m4 // 2][:]
                    dst = bass.AP(
                        ob.tensor, ob.offset + N * (m4 % 2),
                        [[ob.ap[0][0], P], [1, 2], [2, H]],
                    )
                    nc.vector.tensor_copy(dst, src)
                    if i == IMGS - 1:  # tail: store each window immediately
                        eng = nc.scalar if m4 < 2 else nc.sync
                        eng.dma_start(
                            oa[i][P * m4 : P * (m4 + 1), :],
                            o_half[m4 // 2][:, N * (m4 % 2) : N * (m4 % 2) + N],
                        )
                    elif m4 % 2 == 1:  # store half-image once both windows landed
                        mh = m4 // 2
                        nc.scalar.dma_start(
                            oa[i][2 * P * mh : 2 * P * (mh + 1), :].rearrange(
                                "(c p) f -> p c f", p=P
                            ),
                            o_half[mh][:].rearrange("p (c f) -> p c f", c=2),
                        )
                st.pop("p1", None)

            # software-pipelined main loop: folds lead pass A by one image,
            # pass A leads pass B by one image, so the PE stream for image
            # i-2's pass B covers image i-1's pass-A PSUM drain latency.
            load(0)
            folds(0)
            load(1)
            folds(1)
            passA(0)
            for i in range(2, IMGS):
                load(i)
                folds(i)
                passA(i - 1)
                passB(i - 2)
            passA(IMGS - 1)
            passB(IMGS - 2)
            passB(IMGS - 1)

    nc.compile()
    return nc


_NC_CACHE: bacc.Bacc | None = None


def _get_nc() -> bacc.Bacc:
    global _NC_CACHE
    if _NC_CACHE is None:
        _NC_CACHE = _build_nc()
    return _NC_CACHE


def run(inp: np.ndarray, **spmd_kwargs):
    """Shard, run on 8 cores, gather. Returns (output, BassKernelResults)."""
    x = np.asarray(inp, dtype=np.float32)
    assert x.shape == (B, CH, N, N), x.shape
    shards = x.reshape(N_CORES, IMGS, N, N)
    dct_t, deo = _consts()
    in_maps = [
        {"inp": np.ascontiguousarray(shards[c]), "dct_t": dct_t, "deo_t": deo}
        for c in range(N_CORES)
    ]
    res = run_bass_kernel_spmd(_get_nc(), in_maps, core_ids=list(range(N_CORES)), **spmd_kwargs)
    out = np.stack([res.results[c]["out"] for c in range(N_CORES)])
    return out.reshape(B, CH, N, N), res


def kernel(inp: np.ndarray) -> np.ndarray:
    out, _ = run(inp)
    return out
